# revision 10
# baseline (speedup 1.0000x reference)
"""Multi-head causal attention (B=4, S=2048, H=16, Dh=64, Dm=1024) on 8
Trainium2 NeuronCores.

Sharding: core c handles batch b = c//2 and heads [8*(c%2), 8*(c%2)+8).
Each core computes its 8 heads' full attention + O-projection partial sum;
the host adds the two half-head partials per batch plus O_b.

v2 (all matmul operands bf16, PSUM accumulation fp32):
  - software-pipelined: QK projection of pair p+1 (and the O projection,
    for the last pair) is interleaved into the attention inner loop of
    pair p, so the ACT-engine exp stream (the attention bottleneck, ~1.15
    us per 128x1024 tile at 1 elem/lane/cycle) hides under independent PE
    work and the PE stays dense enough to keep the HAM clock gate at 2.4
    GHz.
  - logits per head-PAIR as 2 concurrent row-tiled K=64 matmuls (head A
    in PE rows 0-63, head B in rows 64-127).
  - diagonal k-tiles are N-trimmed: logits/exp/S@V only touch columns
    q >= k-tile start; the 128-wide staircase block is masked by one bf16
    multiply per head (no memset needed - trimmed S@V never reads the
    dead columns).
  - merged S@V (M=65, lhsT = [v_h | 1]) accumulates the softmax
    denominator in PSUM row 64; reciprocal is spread over 128 lanes via a
    DRAM bounce, then broadcast back with a partition-broadcast DMA.
"""

import sys

sys.path.insert(0, "/opt/trn_rl_repo")

import numpy as np
import ml_dtypes

BF16 = ml_dtypes.bfloat16

B, S, DM, H, DH = 4, 2048, 1024, 16, 64
HPC = 8          # heads per core
NPAIR = HPC // 2
PB = 512         # qp block width
NQP = S // PB    # 4 qp blocks
MT = DM // 128   # 8 m-tiles

_cache = {}


def _split_multi_waits(nc, mybir):
    # This container's walrus rejects >1 sync wait per instruction
    # ("Too many sync wait commands").  Move extra waits onto same-engine
    # NoOps right before the instruction; per-engine program order makes
    # this equivalent.
    ctr = 0
    for fn in nc.m.functions:
        for blk in fn.blocks:
            insts = list(blk.instructions)
            new_insts = []
            changed = False
            for inst in insts:
                si = getattr(inst, "sync_info", None)
                waits = list(si.on_wait) if (si is not None and si.on_wait) else []
                if len(waits) > 1:
                    changed = True
                    for w in waits[:-1]:
                        ctr += 1
                        new_insts.append(
                            mybir.InstNoOp(
                                name=f"waitsplit-{ctr}",
                                engine=inst.engine,
                                ins=[],
                                outs=[],
                                sync_info=mybir.SyncInfo(on_wait=[w], on_update=[]),
                            )
                        )
                    si.on_wait = [waits[-1]]
                new_insts.append(inst)
            if changed:
                blk.instructions = new_insts


def _patch_tile_drain(tile_mod, bass_mod):
    # Same walrus limitation hits the Tile kernel-tail drain (one wait per
    # ticked proc).  Chain the waits through single-wait sync NoOps.
    from concourse.vector_clock import ScopedClock, VectorClock

    def _drain_and_barrier(self, tick_clock, wait_clock):
        gc = tick_clock.global_clock
        n = len(gc)
        ticks = [gc[i] for i in range(n)]
        for p in [i for i in range(n) if ticks[i] > 0]:
            nop = self.nc.sync.nop(nofuse=True, hint="drain_wait_split")
            vc = VectorClock([ticks[j] if j == p else 0 for j in range(n)])
            wait_clock.add_sem_waits(nop.ins, ScopedClock({None: vc}))
        self.nc.sync.drain()
        self.nc.all_engine_barrier()
        assert self.sems is not None
        popped = self.nc._tile_sem_poison_stack.pop()
        assert popped is self._sem_poison
        self.nc.clear_and_free_semaphores(list(self.sems.allocated().values()))
        self.nc.all_engine_barrier()

    tile_mod.TileContext._drain_and_barrier = _drain_and_barrier


def _build():
    if "nc" in _cache:
        return _cache["nc"]

    import concourse.bass as bass
    import concourse.mybir as mybir
    import concourse.tile as tile

    _patch_tile_drain(tile, bass)

    f32 = mybir.dt.float32
    bf16 = mybir.dt.bfloat16
    Exp = mybir.ActivationFunctionType.Exp

    nc = bass.Bass()
    xT = nc.dram_tensor("xT", [DM, S], bf16, kind="ExternalInput")
    Wq = nc.dram_tensor("Wq", [DM, 512], bf16, kind="ExternalInput")
    Wk = nc.dram_tensor("Wk", [DM, 512], bf16, kind="ExternalInput")
    Wv = nc.dram_tensor("Wv", [DM, 512], bf16, kind="ExternalInput")
    Wo = nc.dram_tensor("Wo", [512, DM], bf16, kind="ExternalInput")
    qkb = nc.dram_tensor("qkb", [128, 8], f32, kind="ExternalInput")
    vbb = nc.dram_tensor("vbb", [128, 512], f32, kind="ExternalInput")
    msk = nc.dram_tensor("msk", [128, 128], bf16, kind="ExternalInput")
    onz = nc.dram_tensor("onz", [128, 16], bf16, kind="ExternalInput")
    y = nc.dram_tensor("y", [S, DM], bf16, kind="ExternalOutput")
    dd1s = [nc.dram_tensor(f"dd1_{k}", [128, 8], bf16, kind="Internal")
            for k in range(4)]
    dd2s = [nc.dram_tensor(f"dd2_{k}", [128, 8], f32, kind="Internal")
            for k in range(4)]

    with tile.TileContext(nc) as tc:
        with nc.allow_low_precision(reason="bf16 operands feeding the PE"), \
             tc.tile_pool(name="mp", bufs=1) as mp, \
             tc.tile_pool(name="pp", bufs=1, space="PSUM") as pp:
            from contextlib import ExitStack

            # ---- constants ----
            qkb_sb = mp.tile([128, 8], f32, tag="qkb")
            nc.scalar.dma_start(qkb_sb[:], qkb[:])
            vbb_sb = mp.tile([128, 512], f32, tag="vbb")
            nc.scalar.dma_start(vbb_sb[:], vbb[:])
            msk_sb = mp.tile([128, 128], bf16, tag="msk")
            nc.scalar.dma_start(msk_sb[:], msk[:])
            ones_sb = mp.tile([128, 16], bf16, tag="ones")
            nc.scalar.dma_start(ones_sb[:], onz[:])

            # ---- input streams ----
            # xT split into column halves across two DMA queues so the
            # first V-projection matmuls can start ~2us in
            xt = []
            wv = []
            for m in range(MT):
                w = mp.tile([128, 512], bf16, tag=f"wv{m}")
                nc.scalar.dma_start(w[:], Wv[m * 128:(m + 1) * 128, :])
                wv.append(w)
                t = mp.tile([128, S], bf16, tag=f"xt{m}")
                eng = nc.sync if m % 2 == 0 else nc.gpsimd
                eng.dma_start(t[:, 0:1024], xT[m * 128:(m + 1) * 128, 0:1024])
                xt.append(t)
            for m in range(MT):
                eng = nc.sync if m % 2 == 0 else nc.gpsimd
                eng.dma_start(xt[m][:, 1024:2048],
                              xT[m * 128:(m + 1) * 128, 1024:2048])
            wo = []
            for pri in range(NPAIR):
                t = mp.tile([128, DM], bf16, tag=f"wo{pri}")
                nc.scalar.dma_start(t[:], Wo[pri * 128:(pri + 1) * 128, :])
                wo.append(t)

            def emit_wqk_dma(pri):
                tiles = {}
                for ti, W in ((0, Wq), (1, Wk)):
                    for m in range(MT):
                        wt = mp.tile([128, 128], bf16, tag="wqk", bufs=33,
                                     name=f"wqk{ti}_{pri}_{m}")
                        nc.gpsimd.dma_start(
                            wt[:], W[m * 128:(m + 1) * 128,
                                     pri * 128:(pri + 1) * 128])
                        tiles[(ti, m)] = wt
                return tiles

            wqk_next = emit_wqk_dma(0)

            # ---- phase V: value projection, m-major, 2 p-tiles/wave ----
            # waves 0-3 use the "ev" PSUM tag (attention idle then); waves
            # 4-7 run as interleave filler on the serial "proj2" tag.
            # PSUM budget: ev 2x2 + ad 2 + proj2 2 = 8 banks.
            v_sb = [None] * (S // 128)

            def v_wave(wave, tag):
                def run():
                    evt = pp.tile([128, 1024], f32, tag=tag,
                                  bufs=(2 if tag == "ev" else 1),
                                  name=f"vps{wave}")
                    for m in range(MT):
                        for u in range(2):
                            p = 2 * wave + u
                            nc.tensor.matmul(
                                evt[:, u * 512:(u + 1) * 512],
                                xt[m][:, p * 128:(p + 1) * 128], wv[m][:],
                                start=(m == 0), stop=(m == MT - 1))
                    for u in range(2):
                        p = 2 * wave + u
                        vt = mp.tile([128, 520], bf16, tag=f"v{p}")
                        nc.vector.tensor_add(
                            vt.rearrange("p (h c) -> p h c", c=65)[:, :, 0:64],
                            evt[:, u * 512:(u + 1) * 512].rearrange(
                                "p (h c) -> p h c", c=64),
                            vbb_sb.rearrange("p (h c) -> p h c", c=64))
                        nc.vector.tensor_copy(
                            vt.rearrange("p (h c) -> p h c", c=65)
                            [:, :, 64:65],
                            ones_sb[:, 0:8].rearrange("p (h c) -> p h c",
                                                      c=1))
                        v_sb[p] = vt
                return run

            # ---- projection work units (interleave filler) ----
            # weight-stationary: consecutive matmuls share the same lhsT so
            # the LDWEIGHTS cost amortizes over 2 matmuls.
            qkT = {}

            def qk_units(pri, wtiles):
                # 8 units; a (pb-pair, type) group = 2 units of 8 matmuls
                # accumulating into one [128,1024] psum (bufs=1, serial)
                units = []
                for g in range(2):
                    for ti, tname in ((0, "q"), (1, "k")):
                        state = {}
                        def mk(half, g=g, ti=ti, tname=tname, state=state,
                               wtiles=wtiles, pri=pri):
                            def run():
                                if "ps" not in state:
                                    state["ps"] = pp.tile(
                                        [128, 1024], f32, tag="proj2",
                                        bufs=1, name=f"psq{pri}_{ti}{g}")
                                    if (tname, pri) not in qkT:
                                        qkT[(tname, pri)] = mp.tile(
                                            [128, S], bf16, tag=f"{tname}T",
                                            bufs=2, name=f"{tname}T{pri}")
                                ps = state["ps"]
                                for m in range(4 * half, 4 * half + 4):
                                    for u in range(2):
                                        pb = 2 * g + u
                                        nc.tensor.matmul(
                                            ps[:, u * 512:(u + 1) * 512],
                                            wtiles[(ti, m)][:],
                                            xt[m][:,
                                                  pb * 512:(pb + 1) * 512],
                                            start=(m == 0), stop=(m == 7))
                                if half == 1:
                                    out = qkT[(tname, pri)]
                                    for u in range(2):
                                        pb = 2 * g + u
                                        nc.vector.tensor_scalar_add(
                                            out[:, pb * 512:(pb + 1) * 512],
                                            ps[:, u * 512:(u + 1) * 512],
                                            qkb_sb[:, 4 * ti + pri:
                                                   4 * ti + pri + 1])
                            return run
                        units += [mk(0), mk(1)]
                return units

            at_sb = {}
            _ycpy = [0]

            def o_units(i):
                # 4 units; each = one q-subtile pt: 8 matmuls (at-slice
                # stationary across the 2 dm halves) + 2 y copies/DMAs
                units = []
                for pt in range(4):
                    def run(pt=pt, i=i):
                        pso = pp.tile([128, 1024], f32, tag="proj2", bufs=1,
                                      name=f"pso{i}_{pt}")
                        for pri in range(NPAIR):
                            for dm in range(2):
                                nc.tensor.matmul(
                                    pso[:, dm * 512:(dm + 1) * 512],
                                    at_sb[(i, pri)][:, pt * 128:(pt + 1) * 128],
                                    wo[pri][:, dm * 512:(dm + 1) * 512],
                                    start=(pri == 0), stop=(pri == NPAIR - 1))
                        P = 4 * i + pt
                        for dm in range(2):
                            yt = mp.tile([128, 512], bf16, tag="y", bufs=4,
                                         name="yt")
                            _ycpy[0] += 1
                            if _ycpy[0] % 2 == 0:
                                nc.vector.tensor_copy(
                                    yt[:], pso[:, dm * 512:(dm + 1) * 512])
                            else:
                                nc.scalar.copy(
                                    yt[:], pso[:, dm * 512:(dm + 1) * 512])
                            nc.gpsimd.dma_start(
                                y[P * 128:(P + 1) * 128,
                                  dm * 512:(dm + 1) * 512], yt[:])
                    units.append(run)
                return units

            def denom_chain(i, pri, ad):
                slot = (4 * i + pri) % 4
                adc = mp.tile([65, 1024], bf16, tag="adc", bufs=2)
                nc.vector.tensor_copy(adc[:], ad[:])
                dd1 = dd1s[slot][:, :]
                nc.sync.dma_start(
                    dd1.rearrange("p c -> (p c)").rearrange(
                        "(o f) -> o f", o=1), adc[64:65, :])
                dn = mp.tile([128, 8], bf16, tag="dn", bufs=2)
                nc.sync.dma_start(dn[:], dd1)
                dr = mp.tile([128, 8], f32, tag="dr", bufs=2)
                nc.vector.reciprocal(dr[:], dn[:])
                dd2 = dd2s[slot][:, :]
                nc.sync.dma_start(dd2, dr[:])
                bcs = mp.tile([64, 1024], f32, tag="bcs", bufs=2)
                nc.sync.dma_start(
                    bcs[:],
                    dd2.rearrange("p c -> (p c)").rearrange(
                        "(o f) -> o f", o=1).partition_broadcast(64))
                at = mp.tile([128, 512], bf16, tag="at", bufs=17,
                             name=f"at{i}_{pri}")
                nc.vector.tensor_mul(at[0:64, :], adc[0:64, 0:512],
                                     bcs[:, 0:512])
                tmp = mp.tile([64, 512], bf16, tag="tmp", bufs=2)
                nc.vector.tensor_mul(tmp[:], adc[0:64, 512:1024],
                                     bcs[:, 512:1024])
                nc.sync.dma_start(at[64:128, :], tmp[:])
                at_sb[(i, pri)] = at

            # ---- lead-in: V waves 0-3, then q/k pb0-1 of pair 0 ----
            qk0 = qk_units(0, wqk_next)
            for w in range(4):
                v_wave(w, "ev")()
            for u in qk0[:4]:
                u()

            # ---- attention, software-pipelined by one j-step ----
            pend_sv = None      # S@V of the previous j (waits on its exp)
            pend_denom = None   # denominator chain of the previous i-block
            ngroups = sum(4 * (ii + 1) for ii in range(NQP))  # 40 j-steps

            for pri in range(NPAIR):
                wtiles = wqk_next
                if pri < NPAIR - 1:
                    wqk_next = emit_wqk_dma(pri + 1)
                    fill = qk_units(pri + 1, wqk_next)
                    ftot = 8
                else:
                    fill = []
                    ftot = 8  # O units for blocks 0-1 arrive during pair 3
                front = 0
                if pri == 0:
                    # remaining lead work rides as front-loaded filler: the
                    # last V waves and pair-0's q/k pb2-3
                    lead = [v_wave(4, "proj2"), qk0[4], qk0[5],
                            v_wave(5, "proj2"), qk0[6], qk0[7],
                            v_wave(6, "proj2"), v_wave(7, "proj2")]
                    front = len(lead)
                    fill = lead + fill
                fidx = 0
                gctr = 0
                qT = qkT[("q", pri)]
                kT = qkT[("k", pri)]
                for i in range(NQP):
                    if pri == NPAIR - 1 and i >= 2:
                        fill = fill + o_units(i - 2)
                    kmax = 4 * (i + 1)
                    ad = pp.tile([65, 1024], f32, tag="ad", bufs=1)
                    for j in range(kmax):
                        o = (j - 4 * i) * 128 if j >= 4 * i else 0
                        ev = pp.tile([128, 1024], f32, tag="ev", bufs=2)
                        nc.tensor.matmul(
                            ev[:, o:512],
                            kT[0:64, j * 128:(j + 1) * 128],
                            qT[0:64, i * 512 + o:(i + 1) * 512],
                            start=True, stop=True)
                        nc.tensor.matmul(
                            ev[:, 512 + o:1024],
                            kT[64:128, j * 128:(j + 1) * 128],
                            qT[64:128, i * 512 + o:(i + 1) * 512],
                            start=True, stop=True)
                        sc = mp.tile([128, 1024], bf16, tag="sc", bufs=4)
                        nc.scalar.activation(sc[:, o:1024], ev[:, o:1024],
                                             Exp, scale=0.125)
                        if j >= 4 * i:
                            for h in range(2):
                                cb = h * 512 + o
                                nc.vector.tensor_mul(
                                    sc[:, cb:cb + 128],
                                    sc[:, cb:cb + 128], msk_sb[:, :])
                        if pend_sv is not None:
                            pend_sv()
                        if pend_denom is not None:
                            pend_denom()
                            pend_denom = None
                        if fidx < len(fill) and (
                                fidx < front
                                or (fidx - front) * ngroups
                                <= (gctr - front) * ftot):
                            fill[fidx]()
                            fidx += 1
                        gctr += 1

                        def mk_sv(j=j, o=o, sc=sc, ad=ad, kmax=kmax, pri=pri):
                            def run():
                                st = (j == 0)
                                sp = (j == kmax - 1)
                                vt = v_sb[j]
                                for h in range(2):
                                    lh = 2 * pri + h
                                    nc.tensor.matmul(
                                        ad[0:65, h * 512 + o:h * 512 + 512],
                                        vt[:, lh * 65:lh * 65 + 65],
                                        sc[:, h * 512 + o:h * 512 + 512],
                                        start=st, stop=sp)
                            return run
                        pend_sv = mk_sv()
                    # close the i-block: flush its last S@V, then queue the
                    # denominator chain to be emitted inside the next block
                    pend_sv()
                    pend_sv = None

                    def mk_denom(i=i, pri=pri, ad=ad):
                        def run():
                            denom_chain(i, pri, ad)
                        return run
                    pend_denom = mk_denom()
                # drain leftover fill chunks at pair end
                if pend_denom is not None:
                    pend_denom()
                    pend_denom = None
                while fidx < len(fill):
                    fill[fidx]()
                    fidx += 1
            # tail: O for blocks 2-3; block 3's first matmuls overlap the
            # final denominator chain's DMA latency via the block-2 units
            for u in o_units(2) + o_units(3):
                u()

    _split_multi_waits(nc, mybir)
    _cache["nc"] = nc
    return nc


def _host_inputs(x, Q_w, Q_b, K_w, K_b, V_w, V_b, O_w):
    stair = (np.arange(128)[:, None] <= np.arange(128)[None, :]).astype(
        np.float32)
    in_maps = []
    for c in range(8):
        b, hs = c // 2, HPC * (c % 2)
        he = hs + HPC
        qb = Q_b[hs:he].reshape(512).astype(np.float32)
        kb = K_b[hs:he].reshape(512).astype(np.float32)
        qkbm = np.zeros((128, 8), np.float32)
        for pri in range(NPAIR):
            qkbm[:, pri] = qb[pri * 128:(pri + 1) * 128]
            qkbm[:, 4 + pri] = kb[pri * 128:(pri + 1) * 128]
        in_maps.append({
            "xT": np.ascontiguousarray(x[b].T).astype(BF16),
            "Wq": np.ascontiguousarray(
                Q_w[hs:he].transpose(1, 0, 2).reshape(DM, 512)).astype(BF16),
            "Wk": np.ascontiguousarray(
                K_w[hs:he].transpose(1, 0, 2).reshape(DM, 512)).astype(BF16),
            "Wv": np.ascontiguousarray(
                V_w[hs:he].transpose(1, 0, 2).reshape(DM, 512)).astype(BF16),
            "Wo": np.ascontiguousarray(O_w[hs:he].reshape(512, DM)).astype(
                BF16),
            "qkb": qkbm,
            "vbb": np.tile(V_b[hs:he].reshape(1, 512), (128, 1)).astype(
                np.float32),
            "msk": stair.astype(BF16),
            "onz": np.ones((128, 16), BF16),
        })
    return in_maps


def kernel(x, Q_w, Q_b, K_w, K_b, V_w, V_b, O_w, O_b, _trace=False):
    x = np.asarray(x, np.float32)
    args = [np.asarray(a, np.float32)
            for a in (Q_w, Q_b, K_w, K_b, V_w, V_b, O_w)]
    O_b = np.asarray(O_b, np.float32)

    nc = _build()
    from concourse.bass_utils import run_bass_kernel_spmd

    in_maps = _host_inputs(x, *args)
    res = run_bass_kernel_spmd(nc, in_maps, core_ids=list(range(8)),
                               trace=_trace)
    _cache["last_result"] = res
    out = np.empty((B, S, DM), np.float32)
    for b in range(B):
        out[b] = (res.results[2 * b]["y"].astype(np.float32)
                  + res.results[2 * b + 1]["y"].astype(np.float32) + O_b)
    return out


if __name__ == "__main__":
    rng = np.random.default_rng(0)
    x = rng.standard_normal((B, S, DM), dtype=np.float32)
    shp = dict(Q_w=(H, DM, DH), Q_b=(H, DH), K_w=(H, DM, DH), K_b=(H, DH),
               V_w=(H, DM, DH), V_b=(H, DH), O_w=(H, DH, DM), O_b=(DM,))
    ins = {k: rng.standard_normal(v, dtype=np.float32) * 0.05
           for k, v in shp.items()}
    out = kernel(x, **ins)
    print("ran", out.shape, out.dtype)


# revision 13
# speedup vs baseline: 1.0100x; 1.0100x over previous
"""Multi-head causal attention (B=4, S=2048, H=16, Dh=64, Dm=1024) on 8
Trainium2 NeuronCores.

Sharding: core c handles batch b = c//2 and heads [8*(c%2), 8*(c%2)+8).
Each core computes its 8 heads' full attention + O-projection partial sum;
the host adds the two half-head partials per batch plus O_b.

v2 (all matmul operands bf16, PSUM accumulation fp32):
  - software-pipelined: QK projection of pair p+1 (and the O projection,
    for the last pair) is interleaved into the attention inner loop of
    pair p, so the ACT-engine exp stream (the attention bottleneck, ~1.15
    us per 128x1024 tile at 1 elem/lane/cycle) hides under independent PE
    work and the PE stays dense enough to keep the HAM clock gate at 2.4
    GHz.
  - logits per head-PAIR as 2 concurrent row-tiled K=64 matmuls (head A
    in PE rows 0-63, head B in rows 64-127).
  - diagonal k-tiles are N-trimmed: logits/exp/S@V only touch columns
    q >= k-tile start; the 128-wide staircase block is masked by one bf16
    multiply per head (no memset needed - trimmed S@V never reads the
    dead columns).
  - merged S@V (M=65, lhsT = [v_h | 1]) accumulates the softmax
    denominator in PSUM row 64; reciprocal is spread over 128 lanes via a
    DRAM bounce, then broadcast back with a partition-broadcast DMA.
"""

import sys

sys.path.insert(0, "/opt/trn_rl_repo")

import numpy as np
import ml_dtypes

BF16 = ml_dtypes.bfloat16

B, S, DM, H, DH = 4, 2048, 1024, 16, 64
HPC = 8          # heads per core
NPAIR = HPC // 2
PB = 512         # qp block width
NQP = S // PB    # 4 qp blocks
MT = DM // 128   # 8 m-tiles

_cache = {}


def _split_multi_waits(nc, mybir):
    # This container's walrus rejects >1 sync wait per instruction
    # ("Too many sync wait commands").  Move extra waits onto same-engine
    # NoOps right before the instruction; per-engine program order makes
    # this equivalent.
    ctr = 0
    for fn in nc.m.functions:
        for blk in fn.blocks:
            insts = list(blk.instructions)
            new_insts = []
            changed = False
            for inst in insts:
                si = getattr(inst, "sync_info", None)
                waits = list(si.on_wait) if (si is not None and si.on_wait) else []
                if len(waits) > 1:
                    changed = True
                    for w in waits[:-1]:
                        ctr += 1
                        new_insts.append(
                            mybir.InstNoOp(
                                name=f"waitsplit-{ctr}",
                                engine=inst.engine,
                                ins=[],
                                outs=[],
                                sync_info=mybir.SyncInfo(on_wait=[w], on_update=[]),
                            )
                        )
                    si.on_wait = [waits[-1]]
                new_insts.append(inst)
            if changed:
                blk.instructions = new_insts


def _patch_tile_drain(tile_mod, bass_mod):
    # Same walrus limitation hits the Tile kernel-tail drain (one wait per
    # ticked proc).  Chain the waits through single-wait sync NoOps.
    from concourse.vector_clock import ScopedClock, VectorClock

    def _drain_and_barrier(self, tick_clock, wait_clock):
        gc = tick_clock.global_clock
        n = len(gc)
        ticks = [gc[i] for i in range(n)]
        for p in [i for i in range(n) if ticks[i] > 0]:
            nop = self.nc.sync.nop(nofuse=True, hint="drain_wait_split")
            vc = VectorClock([ticks[j] if j == p else 0 for j in range(n)])
            wait_clock.add_sem_waits(nop.ins, ScopedClock({None: vc}))
        self.nc.sync.drain()
        self.nc.all_engine_barrier()
        assert self.sems is not None
        popped = self.nc._tile_sem_poison_stack.pop()
        assert popped is self._sem_poison
        self.nc.clear_and_free_semaphores(list(self.sems.allocated().values()))
        self.nc.all_engine_barrier()

    tile_mod.TileContext._drain_and_barrier = _drain_and_barrier


def _build():
    if "nc" in _cache:
        return _cache["nc"]

    import concourse.bass as bass
    import concourse.mybir as mybir
    import concourse.tile as tile

    _patch_tile_drain(tile, bass)

    f32 = mybir.dt.float32
    bf16 = mybir.dt.bfloat16
    Exp = mybir.ActivationFunctionType.Exp

    nc = bass.Bass()
    xT = nc.dram_tensor("xT", [DM, S], bf16, kind="ExternalInput")
    Wq = nc.dram_tensor("Wq", [DM, 512], bf16, kind="ExternalInput")
    Wk = nc.dram_tensor("Wk", [DM, 512], bf16, kind="ExternalInput")
    Wv = nc.dram_tensor("Wv", [DM, 512], bf16, kind="ExternalInput")
    Wo = nc.dram_tensor("Wo", [512, DM], bf16, kind="ExternalInput")
    cstf = nc.dram_tensor("cstf", [128, 520], f32, kind="ExternalInput")
    cstb = nc.dram_tensor("cstb", [128, 144], bf16, kind="ExternalInput")
    y = nc.dram_tensor("y", [S, DM], bf16, kind="ExternalOutput")
    dd1s = [nc.dram_tensor(f"dd1_{k}", [128, 8], bf16, kind="Internal")
            for k in range(4)]
    dd2s = [nc.dram_tensor(f"dd2_{k}", [128, 8], f32, kind="Internal")
            for k in range(4)]

    with tile.TileContext(nc) as tc:
        with nc.allow_low_precision(reason="bf16 operands feeding the PE"), \
             tc.tile_pool(name="mp", bufs=1) as mp, \
             tc.tile_pool(name="pp", bufs=1, space="PSUM") as pp:
            from contextlib import ExitStack

            # ---- constants (2 merged DMAs: issuance costs ~650ns each) ----
            cstf_sb = mp.tile([128, 520], f32, tag="cstf")
            nc.scalar.dma_start(cstf_sb[:], cstf[:])
            vbb_sb = cstf_sb[:, 0:512]
            qkb_sb = cstf_sb[:, 512:520]
            cstb_sb = mp.tile([128, 144], bf16, tag="cstb")
            nc.scalar.dma_start(cstb_sb[:], cstb[:])
            msk_sb = cstb_sb[:, 0:128]
            ones_sb = cstb_sb[:, 128:144]

            # ---- input streams ----
            # xT is split into per-m column-half tiles across two DMA
            # queues so the first V matmuls start as early as possible;
            # weight streams are merged into few multi-dim-AP transfers.
            xta, xtb = [], []
            for m in range(MT):
                ta = mp.tile([128, 1024], bf16, tag=f"xta{m}")
                tb = mp.tile([128, 1024], bf16, tag=f"xtb{m}")
                eng = nc.sync if m % 2 == 0 else nc.gpsimd
                eng.dma_start(ta[:], xT[m * 128:(m + 1) * 128, 0:1024])
                xta.append(ta)
                xtb.append(tb)
            for m in range(MT):
                eng = nc.sync if m % 2 == 0 else nc.gpsimd
                eng.dma_start(xtb[m][:], xT[m * 128:(m + 1) * 128, 1024:2048])

            def xslice(m, c0, c1):
                if c1 <= 1024:
                    return xta[m][:, c0:c1]
                return xtb[m][:, c0 - 1024:c1 - 1024]

            wvt = []
            for h in range(2):
                w = mp.tile([128, 2048], bf16, tag=f"wv{h}")
                nc.scalar.dma_start(
                    w.rearrange("p (m c) -> p m c", c=512),
                    Wv[h * 512:(h + 1) * 512, :].rearrange(
                        "(m p) c -> p m c", m=4))
                wvt.append(w)

            def wvs(m):
                return wvt[m // 4][:, (m % 4) * 512:(m % 4 + 1) * 512]

            wo_all = mp.tile([128, 4096], bf16, tag="wo")
            nc.scalar.dma_start(
                wo_all.rearrange("p (m c) -> p m c", c=1024),
                Wo.rearrange("(m p) c -> p m c", m=4))

            def emit_wqk_dma(pri):
                tiles = {}
                for ti, W in ((0, Wq), (1, Wk)):
                    wt = mp.tile([128, 1024], bf16, tag="wqk", bufs=5,
                                 name=f"wqk{ti}_{pri}")
                    nc.gpsimd.dma_start(
                        wt.rearrange("p (m c) -> p m c", c=128),
                        W[:, pri * 128:(pri + 1) * 128].rearrange(
                            "(m p) c -> p m c", m=8))
                    for m in range(MT):
                        tiles[(ti, m)] = wt[:, m * 128:(m + 1) * 128]
                return tiles

            wqk_next = emit_wqk_dma(0)

            # ---- phase V: value projection, m-major, 2 p-tiles/wave ----
            # waves 0-3 use the "ev" PSUM tag (attention idle then); waves
            # 4-7 run as interleave filler on the serial "proj2" tag.
            # PSUM budget: ev 2x2 + ad 2 + proj2 2 = 8 banks.
            v_sb = [None] * (S // 128)

            def v_wave(wave, tag):
                def run():
                    evt = pp.tile([128, 1024], f32, tag=tag,
                                  bufs=(2 if tag == "ev" else 1),
                                  name=f"vps{wave}")
                    for m in range(MT):
                        for u in range(2):
                            p = 2 * wave + u
                            nc.tensor.matmul(
                                evt[:, u * 512:(u + 1) * 512],
                                xslice(m, p * 128, (p + 1) * 128), wvs(m),
                                start=(m == 0), stop=(m == MT - 1))
                    for u in range(2):
                        p = 2 * wave + u
                        vt = mp.tile([128, 520], bf16, tag=f"v{p}")
                        nc.vector.tensor_add(
                            vt.rearrange("p (h c) -> p h c", c=65)[:, :, 0:64],
                            evt[:, u * 512:(u + 1) * 512].rearrange(
                                "p (h c) -> p h c", c=64),
                            vbb_sb.rearrange("p (h c) -> p h c", c=64))
                        nc.vector.tensor_copy(
                            vt.rearrange("p (h c) -> p h c", c=65)
                            [:, :, 64:65],
                            ones_sb[:, 0:8].rearrange("p (h c) -> p h c",
                                                      c=1))
                        v_sb[p] = vt
                return run

            # ---- projection work units (interleave filler) ----
            # weight-stationary: consecutive matmuls share the same lhsT so
            # the LDWEIGHTS cost amortizes over 2 matmuls.
            qkT = {}

            def qk_units(pri, wtiles):
                # 8 units; a (pb-pair, type) group = 2 units of 8 matmuls
                # accumulating into one [128,1024] psum (bufs=1, serial)
                units = []
                for g in range(2):
                    for ti, tname in ((0, "q"), (1, "k")):
                        state = {}
                        def mk(half, g=g, ti=ti, tname=tname, state=state,
                               wtiles=wtiles, pri=pri):
                            def run():
                                if "ps" not in state:
                                    state["ps"] = pp.tile(
                                        [128, 1024], f32, tag="proj2",
                                        bufs=1, name=f"psq{pri}_{ti}{g}")
                                    if (tname, pri) not in qkT:
                                        qkT[(tname, pri)] = mp.tile(
                                            [128, S], bf16, tag=f"{tname}T",
                                            bufs=2, name=f"{tname}T{pri}")
                                ps = state["ps"]
                                for m in range(4 * half, 4 * half + 4):
                                    for u in range(2):
                                        pb = 2 * g + u
                                        nc.tensor.matmul(
                                            ps[:, u * 512:(u + 1) * 512],
                                            wtiles[(ti, m)],
                                            xslice(m, pb * 512,
                                                   (pb + 1) * 512),
                                            start=(m == 0), stop=(m == 7))
                                if half == 1:
                                    out = qkT[(tname, pri)]
                                    for u in range(2):
                                        pb = 2 * g + u
                                        nc.vector.tensor_scalar_add(
                                            out[:, pb * 512:(pb + 1) * 512],
                                            ps[:, u * 512:(u + 1) * 512],
                                            qkb_sb[:, 4 * ti + pri:
                                                   4 * ti + pri + 1])
                            return run
                        units += [mk(0), mk(1)]
                return units

            at_sb = {}
            _ycpy = [0]

            def o_units(i, tag="proj2"):
                # 4 units; each = one q-subtile pt: 8 matmuls (at-slice
                # stationary across the 2 dm halves) + 2 y copies, 1 DMA
                units = []
                for pt in range(4):
                    def run(pt=pt, i=i, tag=tag):
                        pso = pp.tile([128, 1024], f32, tag=tag,
                                      bufs=(1 if tag == "proj2" else 2),
                                      name=f"pso{i}_{pt}")
                        for pri in range(NPAIR):
                            for dm in range(2):
                                nc.tensor.matmul(
                                    pso[:, dm * 512:(dm + 1) * 512],
                                    at_sb[(i, pri)][:, pt * 128:(pt + 1) * 128],
                                    wo_all[:, pri * 1024 + dm * 512:
                                           pri * 1024 + dm * 512 + 512],
                                    start=(pri == 0), stop=(pri == NPAIR - 1))
                        P = 4 * i + pt
                        yt = mp.tile([128, 1024], bf16, tag="y", bufs=4,
                                     name="yt")
                        for dm in range(2):
                            _ycpy[0] += 1
                            if _ycpy[0] % 2 == 0:
                                nc.vector.tensor_copy(
                                    yt[:, dm * 512:(dm + 1) * 512],
                                    pso[:, dm * 512:(dm + 1) * 512])
                            else:
                                nc.scalar.copy(
                                    yt[:, dm * 512:(dm + 1) * 512],
                                    pso[:, dm * 512:(dm + 1) * 512])
                        nc.gpsimd.dma_start(
                            y[P * 128:(P + 1) * 128, :], yt[:])
                    units.append(run)
                return units

            def denom_chain(i, pri, ad):
                slot = (4 * i + pri) % 4
                adc = mp.tile([65, 1024], bf16, tag="adc", bufs=2)
                nc.vector.tensor_copy(adc[:], ad[:])
                dd1 = dd1s[slot][:, :]
                nc.sync.dma_start(
                    dd1.rearrange("p c -> (p c)").rearrange(
                        "(o f) -> o f", o=1), adc[64:65, :])
                dn = mp.tile([128, 8], bf16, tag="dn", bufs=2)
                nc.gpsimd.dma_start(dn[:], dd1)
                dr = mp.tile([128, 8], f32, tag="dr", bufs=2)
                nc.vector.reciprocal(dr[:], dn[:])
                dd2 = dd2s[slot][:, :]
                nc.sync.dma_start(dd2, dr[:])
                bcs = mp.tile([64, 1024], f32, tag="bcs", bufs=2)
                nc.gpsimd.dma_start(
                    bcs[:],
                    dd2.rearrange("p c -> (p c)").rearrange(
                        "(o f) -> o f", o=1).partition_broadcast(64))
                at = mp.tile([128, 512], bf16, tag="at", bufs=17,
                             name=f"at{i}_{pri}")
                nc.vector.tensor_mul(at[0:64, :], adc[0:64, 0:512],
                                     bcs[:, 0:512])
                tmp = mp.tile([64, 512], bf16, tag="tmp", bufs=2)
                nc.vector.tensor_mul(tmp[:], adc[0:64, 512:1024],
                                     bcs[:, 512:1024])
                nc.sync.dma_start(at[64:128, :], tmp[:])
                at_sb[(i, pri)] = at

            # ---- lead-in: V waves 0-3, then q/k pb0-1 of pair 0 ----
            qk0 = qk_units(0, wqk_next)
            for w in range(4):
                v_wave(w, "ev")()
            for u in qk0[:4]:
                u()

            # ---- attention, software-pipelined by one j-step ----
            pend_sv = None      # S@V of the previous j (waits on its exp)
            pend_denom = None   # denominator chain of the previous i-block
            ngroups = sum(4 * (ii + 1) for ii in range(NQP))  # 40 j-steps

            for pri in range(NPAIR):
                wtiles = wqk_next
                if pri < NPAIR - 1:
                    wqk_next = emit_wqk_dma(pri + 1)
                    fill = qk_units(pri + 1, wqk_next)
                    ftot = 8
                else:
                    fill = []
                    ftot = 8  # O units for blocks 0-1 arrive during pair 3
                front = 0
                if pri == 0:
                    # remaining lead work rides as front-loaded filler: the
                    # last V waves and pair-0's q/k pb2-3
                    lead = [v_wave(4, "proj2"), qk0[4], qk0[5],
                            v_wave(5, "proj2"), qk0[6], qk0[7],
                            v_wave(6, "proj2"), v_wave(7, "proj2")]
                    front = len(lead)
                    fill = lead + fill
                fidx = 0
                gctr = 0
                qT = qkT[("q", pri)]
                kT = qkT[("k", pri)]
                for i in range(NQP):
                    if pri == NPAIR - 1 and i >= 2:
                        fill = fill + o_units(i - 2)
                    kmax = 4 * (i + 1)
                    ad = pp.tile([65, 1024], f32, tag="ad", bufs=1)
                    for j in range(kmax):
                        o = (j - 4 * i) * 128 if j >= 4 * i else 0
                        ev = pp.tile([128, 1024], f32, tag="ev", bufs=2)
                        nc.tensor.matmul(
                            ev[:, o:512],
                            kT[0:64, j * 128:(j + 1) * 128],
                            qT[0:64, i * 512 + o:(i + 1) * 512],
                            start=True, stop=True)
                        nc.tensor.matmul(
                            ev[:, 512 + o:1024],
                            kT[64:128, j * 128:(j + 1) * 128],
                            qT[64:128, i * 512 + o:(i + 1) * 512],
                            start=True, stop=True)
                        sc = mp.tile([128, 1024], bf16, tag="sc", bufs=4)
                        nc.scalar.activation(sc[:, o:1024], ev[:, o:1024],
                                             Exp, scale=0.125)
                        if j >= 4 * i:
                            for h in range(2):
                                cb = h * 512 + o
                                nc.vector.tensor_mul(
                                    sc[:, cb:cb + 128],
                                    sc[:, cb:cb + 128], msk_sb[:, :])
                        if pend_sv is not None:
                            pend_sv()
                        if pend_denom is not None:
                            pend_denom()
                            pend_denom = None
                        if fidx < len(fill) and (
                                fidx < front
                                or (fidx - front) * ngroups
                                <= (gctr - front) * ftot):
                            fill[fidx]()
                            fidx += 1
                        gctr += 1

                        def mk_sv(j=j, o=o, sc=sc, ad=ad, kmax=kmax, pri=pri):
                            def run():
                                st = (j == 0)
                                sp = (j == kmax - 1)
                                vt = v_sb[j]
                                for h in range(2):
                                    lh = 2 * pri + h
                                    nc.tensor.matmul(
                                        ad[0:65, h * 512 + o:h * 512 + 512],
                                        vt[:, lh * 65:lh * 65 + 65],
                                        sc[:, h * 512 + o:h * 512 + 512],
                                        start=st, stop=sp)
                            return run
                        pend_sv = mk_sv()
                    # close the i-block: flush its last S@V, then queue the
                    # denominator chain to be emitted inside the next block
                    pend_sv()
                    pend_sv = None

                    def mk_denom(i=i, pri=pri, ad=ad):
                        def run():
                            denom_chain(i, pri, ad)
                        return run
                    pend_denom = mk_denom()
                # drain leftover fill chunks at pair end
                if pend_denom is not None:
                    pend_denom()
                    pend_denom = None
                while fidx < len(fill):
                    fill[fidx]()
                    fidx += 1
            # tail: O for blocks 2-3; block 3's first matmuls overlap the
            # final denominator chain's DMA latency via the block-2 units
            for u in o_units(2, tag="ev") + o_units(3, tag="ev"):
                u()

    _split_multi_waits(nc, mybir)
    _cache["nc"] = nc
    return nc


def _host_inputs(x, Q_w, Q_b, K_w, K_b, V_w, V_b, O_w):
    stair = (np.arange(128)[:, None] <= np.arange(128)[None, :]).astype(
        np.float32)
    in_maps = []
    for c in range(8):
        b, hs = c // 2, HPC * (c % 2)
        he = hs + HPC
        qb = Q_b[hs:he].reshape(512).astype(np.float32)
        kb = K_b[hs:he].reshape(512).astype(np.float32)
        qkbm = np.zeros((128, 8), np.float32)
        for pri in range(NPAIR):
            qkbm[:, pri] = qb[pri * 128:(pri + 1) * 128]
            qkbm[:, 4 + pri] = kb[pri * 128:(pri + 1) * 128]
        cstf = np.concatenate(
            [np.tile(V_b[hs:he].reshape(1, 512), (128, 1)).astype(np.float32),
             qkbm], axis=1)
        cstb = np.concatenate(
            [stair.astype(np.float32), np.ones((128, 16), np.float32)],
            axis=1).astype(BF16)
        in_maps.append({
            "xT": np.ascontiguousarray(x[b].T).astype(BF16),
            "Wq": np.ascontiguousarray(
                Q_w[hs:he].transpose(1, 0, 2).reshape(DM, 512)).astype(BF16),
            "Wk": np.ascontiguousarray(
                K_w[hs:he].transpose(1, 0, 2).reshape(DM, 512)).astype(BF16),
            "Wv": np.ascontiguousarray(
                V_w[hs:he].transpose(1, 0, 2).reshape(DM, 512)).astype(BF16),
            "Wo": np.ascontiguousarray(O_w[hs:he].reshape(512, DM)).astype(
                BF16),
            "cstf": cstf,
            "cstb": cstb,
        })
    return in_maps


def kernel(x, Q_w, Q_b, K_w, K_b, V_w, V_b, O_w, O_b, _trace=False):
    x = np.asarray(x, np.float32)
    args = [np.asarray(a, np.float32)
            for a in (Q_w, Q_b, K_w, K_b, V_w, V_b, O_w)]
    O_b = np.asarray(O_b, np.float32)

    nc = _build()
    from concourse.bass_utils import run_bass_kernel_spmd

    in_maps = _host_inputs(x, *args)
    res = run_bass_kernel_spmd(nc, in_maps, core_ids=list(range(8)),
                               trace=_trace)
    _cache["last_result"] = res
    out = np.empty((B, S, DM), np.float32)
    for b in range(B):
        out[b] = (res.results[2 * b]["y"].astype(np.float32)
                  + res.results[2 * b + 1]["y"].astype(np.float32) + O_b)
    return out


if __name__ == "__main__":
    rng = np.random.default_rng(0)
    x = rng.standard_normal((B, S, DM), dtype=np.float32)
    shp = dict(Q_w=(H, DM, DH), Q_b=(H, DH), K_w=(H, DM, DH), K_b=(H, DH),
               V_w=(H, DM, DH), V_b=(H, DH), O_w=(H, DH, DM), O_b=(DM,))
    ins = {k: rng.standard_normal(v, dtype=np.float32) * 0.05
           for k, v in shp.items()}
    out = kernel(x, **ins)
    print("ran", out.shape, out.dtype)
